# revision 40
# baseline (speedup 1.0000x reference)
"""Windowed local self-attention (CrossAttention with the context-overwrite
bug reproduced) on 8 Trainium2 NeuronCores — bf16 rewrite.

Full-input contract: kernel(**inputs) takes unsharded tensors, returns the
full (4, 4096, 1024) fp32 output. The 64 independent 256-token windows are
data-parallel sharded 8 per core; all four projection weights are broadcast.
No collectives.

Key structure (vs the fp32r baseline):
  * All matmul operands are bf16 (tolerance 2e-2; measured ~4e-3). bf16
    enables compiler-automatic FWL (2x faster LDWEIGHTS, hidden by the PE's
    64-deep reorder window).
  * X^T is laid out on the host during shard prep (per-pair transpose +
    bf16 cast), so the device streams it with plain contiguous DMAs — no PE
    transposes, no DVE fixup copies, no XBAR.
  * Windows are processed in pairs so the projection matmuls stream N=512
    moving columns per instruction (half the instruction count).
  * Attention uses PE tile_position concurrency: sim row-tiles the two heads
    of an o-tile (K=64 each at row groups 0/64), S (ones-matmul row sums) and
    AV col-tile the same two heads (M=64 at col groups 0/64). Auto-derived
    from base partitions.
  * Software pipeline over pairs: iteration p issues proj(p) interleaved
    with attention(p-1) and the output projection Y(p-2), keeping the PE
    dense (HAM stays at K=8/8) while exp() runs on the ACT engine.

Per-pair layouts (i = pair-local token 0..511, ws = i//256):
  xt  [128, 8*512]  bf16  block kt: X^T[d in kt-tile, i]
  qT  [128, 8*512]  bf16  block ot: (XWq)^T[o in ot-tile, i]   (kT same)
  v   [128, 4*1024] bf16  block jj=(ws,jt): V[j in jj-tile, o]
  o2T [128, 8*512]  bf16  block ot: normalized attention output^T
"""

import numpy as np
import ml_dtypes

import concourse.bass as bass
import concourse.mybir as mybir
import concourse.tile as tile
from concourse import bacc, bass_utils
from concourse.bass_interp import get_hw_module

H = 16
DH = 64
WIN = 256
D = 1024
B = 4
N = 4096
N_CORES = 8
N_WIN_TOTAL = B * N // WIN          # 64
N_WIN = N_WIN_TOTAL // N_CORES      # 8 windows per core
N_PAIR = N_WIN // 2                 # 4 window-pairs per core
TOK = N_WIN * WIN                   # 2048 token rows per core
PTOK = 2 * WIN                      # 512 tokens per pair
SCALE = DH ** -0.5

F32 = mybir.dt.float32
BF16 = mybir.dt.bfloat16


def _body(tc, xq, wq, wk, wv, wo, out, n_pair):
    nc = tc.nc
    from contextlib import ExitStack

    with ExitStack() as ctx:
        singles = ctx.enter_context(tc.tile_pool(name="singles", bufs=1))
        xpool = ctx.enter_context(tc.tile_pool(name="xpool", bufs=2))
        acts = ctx.enter_context(tc.tile_pool(name="acts", bufs=2))
        heads = ctx.enter_context(tc.tile_pool(name="heads", bufs=8))
        psP = ctx.enter_context(tc.tile_pool(name="psP", bufs=2, space="PSUM"))
        psS = ctx.enter_context(tc.tile_pool(name="psS", bufs=4, space="PSUM"))
        psV = ctx.enter_context(tc.tile_pool(name="psV", bufs=2, space="PSUM"))

        # xq is pre-transposed per pair on the host: row p*1024 + o holds
        # X^T[o, i] for pair p (o = global d, i = pair-local token)
        def alloc_xt(p):
            t = xpool.tile([128, 8 * PTOK], BF16, tag="xt", name=f"xt_{p}")
            for kt in range(8):
                r0 = p * D + kt * 128
                nc.sync.dma_start(
                    t[:, kt * PTOK:(kt + 1) * PTOK], xq[r0:r0 + 128, :]
                )
            return t

        # startup: the first qkT unit needs xt(0)[kt] + wq[kt] for each kt in
        # order — interleave those pairs across both HWDGE queues so the
        # first unit's operands land together and earliest
        wsb = {}
        for name in ("wq", "wk", "wv", "wo"):
            wsb[name] = singles.tile([128, 8 * D], BF16, tag=name, name=f"sb_{name}")
        xt_cur = xpool.tile([128, 8 * PTOK], BF16, tag="xt", name="xt_0")
        for kt in range(8):
            e0, e1 = (nc.sync, nc.scalar) if kt % 2 == 0 else (nc.scalar, nc.sync)
            e0.dma_start(
                xt_cur[:, kt * PTOK:(kt + 1) * PTOK], xq[kt * 128:kt * 128 + 128, :]
            )
            e1.dma_start(
                wsb["wq"][:, kt * D:(kt + 1) * D], wq[kt * 128:(kt + 1) * 128, :]
            )
        for name, w in (("wk", wk), ("wv", wv), ("wo", wo)):
            t = wsb[name]
            for kt in range(8):
                eng = nc.scalar if kt % 2 else nc.sync
                eng.dma_start(
                    t[:, kt * D:(kt + 1) * D], w[kt * 128:(kt + 1) * 128, :]
                )

        ones_f = singles.tile([128, 64], F32)
        nc.gpsimd.memset(ones_f[:], 1.0)
        ones64 = singles.tile([128, 64], BF16)
        nc.vector.tensor_copy(ones64[:], ones_f[:])



        # ================= emission helpers =================

        def emit_qkT_unit(p, dst, wname, ot):
            pq = psP.tile([128, PTOK], F32, tag="acc", name=f"pq_{p}_{wname}_{ot}")
            wtile = wsb[wname]
            for kt in range(8):
                nc.tensor.matmul(
                    pq[:],
                    wtile[:, kt * D + ot * 128:kt * D + (ot + 1) * 128],
                    xts[p][:, kt * PTOK:(kt + 1) * PTOK],
                    start=(kt == 0),
                    stop=(kt == 7),
                )
            nc.vector.tensor_copy(dst[:, ot * PTOK:(ot + 1) * PTOK], pq[:])

        def emit_v_unit(p, v_sb, jj, oc):
            pv = psP.tile([128, 512], F32, tag="acc", name=f"pv_{p}_{jj}_{oc}")
            for kt in range(8):
                nc.tensor.matmul(
                    pv[:],
                    xts[p][:, kt * PTOK + jj * 128:kt * PTOK + (jj + 1) * 128],
                    wsb["wv"][:, kt * D + oc * 512:kt * D + (oc + 1) * 512],
                    start=(kt == 0),
                    stop=(kt == 7),
                )
            # ACT copy keeps the DVE queue free for rs/mul during attention
            nc.scalar.copy(v_sb[:, jj * D + oc * 512:jj * D + (oc + 1) * 512], pv[:])

        def emit_sim(st, qT, kT, ws, pp):
            """sim for head pair (2pp, 2pp+1) of window ws; row-tiled."""
            col0 = pp * PTOK + ws * WIN
            es_pair = []
            ps_pair = []
            for hh in range(2):          # hh = h % 2, row group hh*64
                r0 = hh * 64
                ps_h = psS.tile([128, 512], F32, tag="sim",
                                name=f"sim_{st}_{hh}")
                ps_pair.append(ps_h)
            for jt in range(2):
                for hh in range(2):
                    r0 = hh * 64
                    nc.tensor.matmul(
                        ps_pair[hh][:, jt * WIN:(jt + 1) * WIN],
                        kT[r0:r0 + 64, col0 + jt * 128:col0 + (jt + 1) * 128],
                        qT[r0:r0 + 64, col0:col0 + WIN],
                        start=True,
                        stop=True,
                    )
            for hh in range(2):
                e = heads.tile([128, 512], BF16, tag="es", name=f"es_{st}_{hh}")
                nc.scalar.activation(
                    e[:], ps_pair[hh][:], mybir.ActivationFunctionType.Exp,
                    scale=SCALE,
                )
                es_pair.append(e)
            return es_pair

        def emit_sav(st, es_pair, v_sb, o2T, ws, pp):
            """row-sum S + AV for head pair; col-tiled; writes o2T block."""
            col0 = pp * PTOK + ws * WIN
            sav = psV.tile([128, 2 * WIN], F32, tag="sav", name=f"sav_{st}")
            s_ps = sav[:, 0:WIN]
            av_ps = sav[:, WIN:2 * WIN]
            for jt in range(2):
                for hh in range(2):
                    nc.tensor.matmul(
                        s_ps[hh * 64:(hh + 1) * 64, :],
                        ones64[:],
                        es_pair[hh][:, jt * WIN:(jt + 1) * WIN],
                        start=(jt == 0),
                        stop=(jt == 1),
                        skip_group_check=True,
                    )
            for jt in range(2):
                jj = ws * 2 + jt
                for hh in range(2):
                    h = 2 * pp + hh
                    nc.tensor.matmul(
                        av_ps[hh * 64:(hh + 1) * 64, :],
                        v_sb[:, jj * D + h * DH:jj * D + (h + 1) * DH],
                        es_pair[hh][:, jt * WIN:(jt + 1) * WIN],
                        start=(jt == 0),
                        stop=(jt == 1),
                        skip_group_check=True,
                    )
            rs = heads.tile([128, WIN], F32, tag="rs", name=f"rs_{st}", bufs=2)
            nc.vector.reciprocal_approx_fast(rs[:], s_ps[:])
            nc.vector.tensor_mul(o2T[:, col0:col0 + WIN], av_ps[:], rs[:])

        def emit_y_group(p, o2T, it, ec):
            py = psP.tile([128, 512], F32, tag="acc", name=f"py_{p}_{it}_{ec}")
            for ot in range(8):
                nc.tensor.matmul(
                    py[:],
                    o2T[:, ot * PTOK + it * 128:ot * PTOK + (it + 1) * 128],
                    wsb["wo"][:, ot * D + ec * 512:ot * D + (ec + 1) * 512],
                    start=(ot == 0),
                    stop=(ot == 7),
                )
            y_sb = acts.tile([128, 512], F32, tag="y", name=f"y_{p}_{it}_{ec}", bufs=3)
            nc.vector.tensor_copy(y_sb[:], py[:])
            r0 = p * PTOK + it * 128
            eng = nc.sync if (it + ec) % 2 == 0 else nc.scalar
            eng.dma_start(out[r0:r0 + 128, ec * 512:(ec + 1) * 512], y_sb[:])

        # ================= software pipeline =================

        xts = {0: xt_cur}
        pair_tiles = {}   # p -> (qT, kT, v_sb, o2T)
        ATT_STEPS = [(ws, pp) for ws in range(2) for pp in range(8)]
        N_INJ = 8         # last pair's attention steps injected pre-drain
        inj_pending = None

        def attn_step_sim(p, st):
            ws, pp = ATT_STEPS[st]
            qT, kT, v_sb, o2T = pair_tiles[p]
            return emit_sim(f"{p}_{st}", qT, kT, ws, pp)

        def attn_step_sav(p, st, es_pair):
            ws, pp = ATT_STEPS[st]
            qT, kT, v_sb, o2T = pair_tiles[p]
            emit_sav(f"{p}_{st}", es_pair, v_sb, o2T, ws, pp)

        for p in range(n_pair):
            if p + 1 < n_pair:
                xts[p + 1] = alloc_xt(p + 1)

            qT = acts.tile([128, 8 * PTOK], BF16, tag="qT", name=f"qT_{p}")
            kT = acts.tile([128, 8 * PTOK], BF16, tag="kT", name=f"kT_{p}")
            v_sb = acts.tile([128, 4 * D], BF16, tag="v", name=f"v_{p}")
            o2T = acts.tile([128, 8 * PTOK], BF16, tag="o2T", name=f"o2T_{p}")
            pair_tiles[p] = (qT, kT, v_sb, o2T)

            proj_units = (
                [lambda ot=ot: emit_qkT_unit(p, qT, "wq", ot) for ot in range(8)]
                + [lambda ot=ot: emit_qkT_unit(p, kT, "wk", ot) for ot in range(8)]
                + [lambda jj=jj, oc=oc: emit_v_unit(p, v_sb, jj, oc)
                   for jj in range(4) for oc in range(2)]
            )
            # Y of pair q is split: window-0 groups (it 0-1) become ready
            # mid-way through attn(q) and run late in iteration q+1; the
            # window-1 groups run in iteration q+2
            y_units = []
            if p >= 2:
                o2T_y = pair_tiles[p - 2][3]
                y_units = [
                    lambda it=it, ec=ec: emit_y_group(p - 2, o2T_y, it, ec)
                    for it in range(2, 4) for ec in range(2)
                ]

            if p == 0:
                for u in proj_units:
                    u()
            else:
                # interleave: 16 attention steps of pair p-1, one-step
                # sim->sav delay, filler = proj units of p + Y-ws1 of p-2 +
                # (late) Y-ws0 of p-1.
                # [sim, sav, F, F] order: attention MMs stay adjacent, so the
                # full-array LDWEIGHTS stall at each attn<->proj transition is
                # paid twice per step instead of four times.
                filler = proj_units + y_units
                o2T_prev = pair_tiles[p - 1][3]
                fi = 0
                pending = None
                late_y = []
                last_iter = (p == n_pair - 1)
                for st in range(16):
                    es_pair = attn_step_sim(p - 1, st)
                    if pending is not None:
                        attn_step_sav(p - 1, pending[0], pending[1])
                    pending = (st, es_pair)
                    if st == 9:
                        # window 0 of pair p-1 fully normalized after step
                        # 7's sav (emitted during st=8)
                        late_y = [
                            lambda it=it, ec=ec: emit_y_group(p - 1, o2T_prev, it, ec)
                            for it in range(2) for ec in range(2)
                        ]
                    # fillers first: the injected steps below read v-units
                    # that must be emitted before their sav; late_y units
                    # have no downstream readers so they pack the tail slots
                    for _ in range(2):
                        if fi < len(filler):
                            filler[fi](); fi += 1
                        elif late_y:
                            late_y.pop(0)()
                    if last_iter and st >= 16 - N_INJ:
                        # inject the last pair's first attention steps into
                        # this PE-dense region; the ACT-paced drain shrinks
                        s3 = st - (16 - N_INJ)
                        es3 = attn_step_sim(p, s3)
                        if inj_pending is not None:
                            attn_step_sav(p, inj_pending[0], inj_pending[1])
                        inj_pending = (s3, es3)
                attn_step_sav(p - 1, pending[0], pending[1])
                while late_y:
                    late_y.pop(0)()
                while fi < len(filler):
                    filler[fi](); fi += 1

        # ---- drain: attention of last pair + remaining Y groups ----
        last = n_pair - 1
        filler = []
        if n_pair >= 2:
            o2T_y = pair_tiles[n_pair - 2][3]
            filler = [
                lambda it=it, ec=ec: emit_y_group(n_pair - 2, o2T_y, it, ec)
                for it in range(2, 4) for ec in range(2)
            ]
        fi = 0
        pending = inj_pending if n_pair >= 2 else None
        start_st = N_INJ if n_pair >= 2 else 0
        late_y = []
        o2T_last = pair_tiles[last][3]
        for st in range(start_st, 16):
            es_pair = attn_step_sim(last, st)
            if pending is not None:
                attn_step_sav(last, pending[0], pending[1])
            pending = (st, es_pair)
            if st == 9:
                late_y = [
                    lambda it=it, ec=ec: emit_y_group(last, o2T_last, it, ec)
                    for it in range(2) for ec in range(2)
                ]
            # 1 filler per step: spreads the available units over the
            # ACT-paced drain steps instead of exhausting them early
            if late_y:
                late_y.pop(0)()
            elif fi < len(filler):
                filler[fi](); fi += 1
        attn_step_sav(last, pending[0], pending[1])
        while fi < len(filler):
            filler[fi](); fi += 1
        for it in range(2, 4):
            for ec in range(2):
                emit_y_group(last, o2T_last, it, ec)


_CACHE = {}


def _build(n_pair=N_PAIR):
    key = n_pair
    if key in _CACHE:
        return _CACHE[key]
    tok = n_pair * PTOK
    nc = bacc.Bacc(
        "TRN2", target_bir_lowering=False, debug=False, num_devices=N_CORES
    )
    xq = nc.dram_tensor("xq", [n_pair * D, PTOK], BF16, kind="ExternalInput").ap()
    wq = nc.dram_tensor("Wq", [D, D], BF16, kind="ExternalInput").ap()
    wk = nc.dram_tensor("Wk", [D, D], BF16, kind="ExternalInput").ap()
    wv = nc.dram_tensor("Wv", [D, D], BF16, kind="ExternalInput").ap()
    wo = nc.dram_tensor("Wo", [D, D], BF16, kind="ExternalInput").ap()
    out = nc.dram_tensor("out", [tok, D], F32, kind="ExternalOutput").ap()
    with tile.TileContext(nc) as tc:
        _body(tc, xq, wq, wk, wv, wo, out, n_pair)
    nc.compile()
    nc.m = get_hw_module(nc.m)
    _CACHE[key] = nc
    return nc


def run(query, Wq, Wk, Wv, Wo, bo, n_pair=N_PAIR, **spmd_kwargs):
    nc = _build(n_pair)
    tok = n_pair * PTOK
    # shard prep: bf16 cast + per-pair transpose so each core streams
    # X^T[d, pair-token] with contiguous 1KB DMA lines
    q2 = (
        np.asarray(query, dtype=np.float32)
        .astype(ml_dtypes.bfloat16)
        .reshape(-1, PTOK, D)
        .transpose(0, 2, 1)      # [n_pair_total, D, PTOK]
        .reshape(-1, PTOK)       # rows: pair-major, then d
    )
    q2 = np.ascontiguousarray(q2)
    weights = {
        "Wq": np.asarray(Wq, np.float32).astype(ml_dtypes.bfloat16),
        "Wk": np.asarray(Wk, np.float32).astype(ml_dtypes.bfloat16),
        "Wv": np.asarray(Wv, np.float32).astype(ml_dtypes.bfloat16),
        "Wo": np.asarray(Wo, np.float32).astype(ml_dtypes.bfloat16),
    }
    in_maps = []
    rows_per_core = N_PAIR * D
    for c in range(N_CORES):
        m = {"xq": q2[c * rows_per_core:c * rows_per_core + n_pair * D]}
        m.update(weights)
        in_maps.append(m)
    res = bass_utils.run_bass_kernel_spmd(
        nc, in_maps, core_ids=list(range(N_CORES)), **spmd_kwargs
    )
    outs = [res.results[c]["out"] for c in range(N_CORES)]
    return outs, res


def kernel(query, context, Wq, Wk, Wv, Wo, bo):
    outs, _ = run(query, Wq, Wk, Wv, Wo, bo)
    y = np.concatenate(outs, axis=0).reshape(B, N, D)
    bo = np.asarray(bo, np.float32)
    if bo.any():
        y = y + bo  # bias is structurally zero for this problem
    return y.astype(np.float32)
